# revision 23
# baseline (speedup 1.0000x reference)
"""HKLinear Trainium2 kernel — 8-core SPMD over tokens, one-level Strassen.

OUT.T = W @ x.T with W [4096f, 4096k], x.T [4096k, 1024t] per core, split
2x2 over (f, k) for W / (k, t) for x.T:
    P1=(A11+A22)(B11+B22) P2=(A21+A22)B11 P3=A11(B12-B22) P4=A22(B21-B11)
    P5=(A11+A12)B22 P6=(A21-A11)(B11+B12) P7=(A12-A22)(B21+B22)
    C11=P1+P4-P5+P7  C12=P3+P5  C21=P2+P4  C22=P1-P2+P3+P6
7/8 of the direct matmul cycles (1792 vs 2048 MMs); W-combos are precomputed
on the host (linear prep of weights), x-combos on the DVE, recombination on
the DVE from PSUM.  Measured rel err 5.7e-3 on hw (gate 2e-2).
"""

import numpy as np
import ml_dtypes

N_CORES = 8
IN_F = 4096
OUT_F = 4096
N_CLUSTERS = 64
THRESHOLD = 0.01
TEMPERATURE = 0.1
N_TOKENS = 8192
TOK_PER_CORE = 1024

KTH = 16          # k-tiles per K-half
NTH = 16          # feature chunks per F-half
P2_HEAD_J = 7     # P2 chunks computed in the head
PARK_J = 4        # j's recombined unmasked (mask applied later)
EXP_SHIFT = -30.0

BF16 = ml_dtypes.bfloat16


def _install_ldw_dedup():
    import concourse.tile as tile
    import concourse.mybir as mybir
    if getattr(tile, "_ldw_dedup_installed", False):
        return
    orig_legalize = tile.tile_legalize

    def ldw_key(i):
        return (str(i.ins[0]), str(i.perf_mode), str(i.tile_position),
                str(i.tile_size), str(i.is_transpose))

    def dedup_legalize(ordered, nc):
        out = orig_legalize(ordered, nc)
        for bb, insts in out.items():
            new, last_key, pending = [], None, None
            for i in insts:
                if getattr(i, "engine", None) != mybir.EngineType.PE:
                    new.append(i)
                    continue
                nm = type(i).__name__
                if nm == "InstLdweights":
                    k = ldw_key(i)
                    if k == last_key:
                        pending = i
                        continue
                    last_key = k
                    new.append(i)
                elif nm == "InstMatmult":
                    if pending is not None:
                        i.merge_dependencies_from(pending)
                        pending = None
                    new.append(i)
                else:
                    last_key, pending = None, None
                    new.append(i)
            out[bb] = new
        return out

    tile.tile_legalize = dedup_legalize
    tile._ldw_dedup_installed = True


def _build_bass():
    _install_ldw_dedup()
    import concourse.bass as bass
    import concourse.mybir as mybir
    import concourse.tile as tile
    from concourse import bacc
    from concourse.bass import ds

    f32 = mybir.dt.float32
    bf16 = mybir.dt.bfloat16
    AT = mybir.AluOpType

    nc = bacc.Bacc("TRN2", target_bir_lowering=False, debug=False,
                   num_devices=N_CORES)

    # W combos M1..M7, packed [i][j][p(k%128)][kt][f]
    wt7_d = nc.dram_tensor("wt7", [7, NTH, 128, KTH, 128], bf16, kind="ExternalInput")
    xb_d = {b: nc.dram_tensor(f"x{b}", [128, KTH, 512], bf16, kind="ExternalInput")
            for b in ("b11", "b12", "b21", "b22")}
    ct_d = nc.dram_tensor("ct", [128, 2 * KTH, N_CLUSTERS], bf16, kind="ExternalInput")
    ac_d = nc.dram_tensor("ac", [N_CLUSTERS, 32, 128], bf16, kind="ExternalInput")
    bc_d = nc.dram_tensor("bc", [128, 32], f32, kind="ExternalInput")
    out_d = nc.dram_tensor("out", [32, 2, 128, 512], f32, kind="ExternalOutput")

    with tile.TileContext(nc) as tc:
        with (
            tc.tile_pool(name="resident", bufs=1) as resident,
            tc.tile_pool(name="xc", bufs=5) as xc,
            tc.tile_pool(name="wpool", bufs=7) as wpool,
            tc.tile_pool(name="p2pool", bufs=P2_HEAD_J) as p2pool,
            tc.tile_pool(name="tmp", bufs=3) as tmp,
            tc.tile_pool(name="cpark", bufs=4 * PARK_J) as cpark,
            tc.tile_pool(name="opool", bufs=2) as opool,
            tc.tile_pool(name="route_sb", bufs=1) as route_sb,
            tc.tile_pool(name="pmain", bufs=7, space="PSUM") as pmain,
            tc.tile_pool(name="psum_scr", bufs=1, space="PSUM") as psum_scr,
            tc.tile_pool(name="cc_dram", bufs=1, space="DRAM") as cc_dram,
        ):
            # ---- DMA order: head W combos, xb11, ct, other x blocks, rest ----
            w2_head = []
            for j in range(5):
                w_sb = wpool.tile([128, KTH, 128], bf16, tag="w_sb",
                                  name=f"w2h_{j}")
                nc.sync.dma_start(w_sb[:], wt7_d[1, j])
                w2_head.append(w_sb)

            xb11 = resident.tile([128, KTH, 512], bf16)
            late_w = {6: 5, 8: 6}  # chunk idx -> head W combo to slip in
            for k in range(KTH):
                nc.sync.dma_start(xb11[:, k], xb_d["b11"][:, k])
                if k in late_w:
                    j = late_w[k]
                    w_sb = wpool.tile([128, KTH, 128], bf16, tag="w_sb",
                                      name=f"w2h_{j}")
                    nc.sync.dma_start(w_sb[:], wt7_d[1, j])
                    w2_head.append(w_sb)
            ct_sb = resident.tile([128, 2 * KTH, N_CLUSTERS], bf16)
            nc.sync.dma_start(ct_sb[:], ct_d[:])
            xb21 = xc.tile([128, KTH, 512], bf16, tag="xc", name="xb21")
            for k in range(KTH):
                nc.sync.dma_start(xb21[:, k], xb_d["b21"][:, k])
            xb12 = xc.tile([128, KTH, 512], bf16, tag="xc", name="xb12")
            for k in range(KTH):
                nc.sync.dma_start(xb12[:, k], xb_d["b12"][:, k])
            xb22 = resident.tile([128, KTH, 512], bf16)
            for k in range(KTH):
                nc.sync.dma_start(xb22[:, k], xb_d["b22"][:, k])
            a_sb = resident.tile([N_CLUSTERS, 32, 128], bf16)
            nc.sync.dma_start(a_sb[:], ac_d[:])
            bc_sb = resident.tile([128, 32], f32)
            nc.sync.dma_start(bc_sb[:], bc_d[:])

            shift_col = route_sb.tile([N_CLUSTERS, 1], f32)
            nc.vector.memset(shift_col[:], EXP_SHIFT)
            ones_c = route_sb.tile([N_CLUSTERS, 1], bf16)
            nc.vector.memset(ones_c[:], 1.0)
            ones_r = route_sb.tile([1, N_CLUSTERS], bf16)
            nc.vector.memset(ones_r[:], 1.0)
            ms_sb = route_sb.tile([128, 32], f32)
            bmask_sb = route_sb.tile([128, 32], f32)

            # ---- head: P2[j] for j<P2_HEAD_J, paced with the b11 stream ----
            p2sb = []
            for jg in (range(0, 5), range(5, P2_HEAD_J)):
                ps = {j: pmain.tile([128, 512], f32, tag="pm", name=f"p2_{j}")
                      for j in jg}
                for k in range(KTH):
                    for j in jg:
                        nc.tensor.matmul(ps[j][:], w2_head[j][:, k, :],
                                         xb11[:, k, :],
                                         start=(k == 0), stop=(k == KTH - 1))
                for j in jg:
                    s_t = p2pool.tile([128, 512], bf16, tag="p2s")
                    nc.scalar.activation(s_t[:], ps[j][:],
                                         mybir.ActivationFunctionType.Copy)
                    p2sb.append(s_t)

            # ---- routing on raw x blocks ----
            psum_ls = [pmain.tile([128, 512], f32, tag="pm", name=f"rl_{h}")
                       for h in range(2)]
            rhs_half = {0: (xb11, xb21), 1: (xb12, xb22)}
            for k in range(2 * KTH):
                for h in range(2):
                    lo, hi = rhs_half[h]
                    rhs = lo[:, k, :] if k < KTH else hi[:, k - KTH, :]
                    nc.tensor.matmul(psum_ls[h][ds(0, N_CLUSTERS), :],
                                     ct_sb[:, k, :], rhs,
                                     start=(k == 0), stop=(k == 2 * KTH - 1))
            cmax_h = []
            for h in range(2):
                pl = psum_ls[h][ds(0, N_CLUSTERS), :]
                e_f = route_sb.tile([N_CLUSTERS, 512], f32, tag="e_f", bufs=1)
                nc.scalar.activation(e_f[:], pl,
                                     mybir.ActivationFunctionType.Exp,
                                     bias=shift_col[:], scale=1.0)
                e_b = route_sb.tile([N_CLUSTERS, 512], bf16, tag="e_b", bufs=1)
                nc.scalar.activation(e_b[:], pl,
                                     mybir.ActivationFunctionType.Exp,
                                     bias=shift_col[:], scale=1.0)
                psum_s = psum_scr.tile([128, 512], f32, tag="scr")
                nc.tensor.matmul(psum_s[ds(0, 1), :], ones_c[:], e_b[:],
                                 start=True, stop=True)
                s_b = route_sb.tile([1, 512], bf16, tag="s_b", bufs=1)
                nc.scalar.activation(s_b[:], psum_s[ds(0, 1), :],
                                     mybir.ActivationFunctionType.Copy,
                                     scale=THRESHOLD)
                psum_b = psum_scr.tile([128, 512], f32, tag="scr")
                nc.tensor.matmul(psum_b[ds(0, N_CLUSTERS), :], ones_r[:], s_b[:],
                                 start=True, stop=True)
                d_sb = route_sb.tile([N_CLUSTERS, 512], f32, tag="d_sb", bufs=1)
                nc.vector.tensor_tensor(d_sb[:], e_f[:],
                                        psum_b[ds(0, N_CLUSTERS), :],
                                        op=AT.subtract)
                cm = route_sb.tile([N_CLUSTERS, 1], f32, tag="cm", bufs=2)
                nc.vector.reduce_max(cm[:], d_sb[:], axis=mybir.AxisListType.X)
                cmax_h.append(cm)
            cmax = route_sb.tile([N_CLUSTERS, 1], f32)
            nc.vector.tensor_tensor(cmax[:], cmax_h[0][:], cmax_h[1][:], op=AT.max)

            cc_in = cc_dram.tile([N_CLUSTERS, 1], f32)
            cc_out = cc_dram.tile([N_CLUSTERS, 1], f32, addr_space="Shared")
            nc.gpsimd.dma_start(cc_in[:], cmax[:])
            nc.gpsimd.collective_compute(
                "AllReduce", AT.max,
                replica_groups=[list(range(N_CORES))],
                ins=[cc_in.opt()], outs=[cc_out.opt()],
            )
            cmax_red = route_sb.tile([N_CLUSTERS, 1], f32)
            nc.gpsimd.dma_start(cmax_red[:], cc_out[:])
            sel_f = route_sb.tile([N_CLUSTERS, 1], f32)
            nc.vector.tensor_scalar(sel_f[:], cmax_red[:], 0.0, None,
                                    op0=AT.is_gt)
            sel_bf = route_sb.tile([N_CLUSTERS, 1], bf16)
            nc.vector.tensor_copy(sel_bf[:], sel_f[:])

            # ---- x combos on DVE.  xc is a 5-slot ring with allocation order
            # xb21, xb12, R1, R4, R7, R3, R6: R3 lands in xb21's slot (its
            # inputs are xb12/xb22) and R6 = R1 + R3 lands in xb12's slot
            # (algebraic identity B11+B12 = (B11+B22) + (B12-B22), avoiding a
            # read of the xb12 memory it overwrites).
            def combo(name, a, b, op):
                t = xc.tile([128, KTH, 512], bf16, tag="xc", name=name)
                for k in range(KTH):
                    nc.vector.tensor_tensor(t[:, k], a[:, k], b[:, k], op=op)
                return t
            R1 = combo("R1", xb11, xb22, AT.add)
            R4 = combo("R4", xb21, xb11, AT.subtract)
            R7 = combo("R7", xb21, xb22, AT.add)
            R3 = combo("R3", xb12, xb22, AT.subtract)
            R6 = combo("R6", R1, R3, AT.add)

            # product index -> (wt7 row, rhs tile)
            prod_rhs = {1: R1, 2: xb11, 3: R3, 4: R4, 5: xb22, 6: R6, 7: R7}

            def emit_product(i, j):
                w_sb = wpool.tile([128, KTH, 128], bf16, tag="w_sb",
                                  name=f"w_{i}_{j}")
                nc.sync.dma_start(w_sb[:], wt7_d[i - 1, j])
                ps = pmain.tile([128, 512], f32, tag="pm", name=f"p_{i}_{j}")
                rhs = prod_rhs[i]
                for k in range(KTH):
                    nc.tensor.matmul(ps[:], w_sb[:, k, :], rhs[:, k, :],
                                     start=(k == 0), stop=(k == KTH - 1))
                return ps

            def stage(ps, name):
                # ACT-drain a product psum to a bf16 SBUF tile (frees the bank)
                s_t = tmp.tile([128, 512], bf16, tag="pstg", bufs=8, name=name)
                nc.scalar.activation(s_t[:], ps[:],
                                     mybir.ActivationFunctionType.Copy)
                return s_t

            def chain(parts, masked_n):
                # parts: (sign, AP) with at most the LAST one living in PSUM
                acc = None
                for sgn, ap in parts:
                    if acc is None:
                        assert sgn > 0
                        acc = ap
                        continue
                    t = tmp.tile([128, 512], f32, tag="t")
                    nc.vector.tensor_tensor(t[:], acc, ap,
                                            op=(AT.add if sgn > 0 else AT.subtract))
                    acc = t[:]
                if masked_n is None:
                    o = cpark.tile([128, 512], bf16, tag="cp")
                    nc.vector.tensor_copy(o[:], acc)
                    return o
                o = opool.tile([128, 512], f32, tag="o_sb")
                nc.vector.tensor_scalar(o[:], acc,
                                        ms_sb[:, ds(masked_n, 1)],
                                        bmask_sb[:, ds(masked_n, 1)],
                                        op0=AT.mult, op1=AT.add)
                return o

            parked = []

            def finish(acc, masked_n):
                if masked_n is None:
                    o = cpark.tile([128, 512], bf16, tag="cp")
                    nc.vector.tensor_copy(o[:], acc)
                    return o
                o = opool.tile([128, 512], f32, tag="o_sb")
                nc.vector.tensor_scalar(o[:], acc,
                                        ms_sb[:, ds(masked_n, 1)],
                                        bmask_sb[:, ds(masked_n, 1)],
                                        op0=AT.mult, op1=AT.add)
                return o

            def tt(a, b, op):
                t = tmp.tile([128, 512], f32, tag="t", bufs=4)
                nc.vector.tensor_tensor(t[:], a, b, op=op)
                return t[:]

            def emit_j(j):
                # chains interleave with the product matmuls: everything that
                # doesn't need P6/P7 is emitted before their MMs, so only one
                # tensor_tensor (+mask) trails the last matmul of the group
                masked = j >= PARK_J
                if j < P2_HEAD_J:
                    p2 = p2sb[j][:]
                else:
                    p2 = stage(emit_product(2, j), f"p2s_{j}")[:]
                p5 = stage(emit_product(5, j), f"p5s_{j}")[:]
                p1 = stage(emit_product(1, j), f"p1s_{j}")[:]
                p4 = stage(emit_product(4, j), f"p4s_{j}")[:]
                n1, n2 = j, NTH + j
                c21 = finish(tt(p2, p4, AT.add), n2 if masked else None)
                t_b = tt(tt(p1, p4, AT.add), p5, AT.subtract)      # c11 partial
                p3 = stage(emit_product(3, j), f"p3s_{j}")[:]
                c12 = finish(tt(p3, p5, AT.add), n1 if masked else None)
                t_d = tt(tt(p1, p2, AT.subtract), p3, AT.add)      # c22 partial
                p6 = emit_product(6, j)[:]   # single-read, stays in PSUM
                c22 = finish(tt(t_d, p6, AT.add), n2 if masked else None)
                p7 = emit_product(7, j)[:]   # single-read, stays in PSUM
                c11 = finish(tt(t_b, p7, AT.add), n1 if masked else None)
                tiles = {(n1, 0): c11, (n2, 0): c21, (n1, 1): c12, (n2, 1): c22}
                if masked:
                    for (n, h), t in tiles.items():
                        nc.sync.dma_start(out_d[n, h], t[:])
                else:
                    parked.append(tiles)

            for j in range(PARK_J):
                emit_j(j)

            # ---- row mask (PE gather), then flush parked chunks ----
            psum_m = psum_scr.tile([128, 512], f32, tag="scr")
            for n in range(32):
                nc.tensor.matmul(psum_m[:, ds(n, 1)], a_sb[:, n, :], sel_bf[:],
                                 start=True, stop=True)
            nc.scalar.activation(ms_sb[:], psum_m[:, ds(0, 32)],
                                 mybir.ActivationFunctionType.Copy)
            nc.vector.tensor_tensor(bmask_sb[:], ms_sb[:], bc_sb[:], op=AT.mult)

            for tiles in parked:
                for (n, h), t in tiles.items():
                    o = opool.tile([128, 512], f32, tag="o_sb")
                    nc.vector.tensor_scalar(o[:], t[:],
                                            ms_sb[:, ds(n, 1)],
                                            bmask_sb[:, ds(n, 1)],
                                            op0=AT.mult, op1=AT.add)
                    nc.sync.dma_start(out_d[n, h], o[:])

            for j in range(PARK_J, NTH):
                emit_j(j)

    nc.compile()
    return nc


_NC_CACHE = None


def _get_nc():
    global _NC_CACHE
    if _NC_CACHE is None:
        _NC_CACHE = _build_bass()
    return _NC_CACHE


def _prep_in_maps(input, weight, bias, centroids, assignments):
    x = np.ascontiguousarray(np.asarray(input, dtype=np.float32).reshape(N_TOKENS, IN_F))
    w = np.asarray(weight, dtype=np.float32)
    b = np.asarray(bias, dtype=np.float32)
    c = np.asarray(centroids, dtype=np.float32)
    a = np.asarray(assignments)

    A11, A12 = w[:2048, :2048], w[:2048, 2048:]
    A21, A22 = w[2048:, :2048], w[2048:, 2048:]
    Ms = [A11 + A22, A21 + A22, A11, A22, A11 + A12, A21 - A11, A12 - A22]

    def pack_m(m):
        # m [2048f, 2048k] -> [NTH, 128p, KTH, 128f]
        return m.reshape(NTH, 128, KTH, 128).transpose(0, 3, 2, 1)
    wt7 = np.ascontiguousarray(np.stack([pack_m(m) for m in Ms])).astype(BF16)

    ct = np.ascontiguousarray(
        (c / TEMPERATURE).T.reshape(2 * KTH, 128, N_CLUSTERS).transpose(1, 0, 2)).astype(BF16)
    ac = (a[None, :] == np.arange(N_CLUSTERS, dtype=a.dtype)[:, None])
    ac = np.ascontiguousarray(ac.reshape(N_CLUSTERS, 32, 128)).astype(BF16)
    bc = np.ascontiguousarray(b.reshape(32, 128).T).astype(np.float32)

    def pack_x(xs, klo, tlo):
        # -> [128p, KTH, 512t]: blk[p, kt, t] = xs[tlo+t, klo + kt*128 + p]
        blk = xs[tlo:tlo + 512, klo:klo + 2048]
        return np.ascontiguousarray(
            blk.T.reshape(KTH, 128, 512).transpose(1, 0, 2)).astype(BF16)

    in_maps = []
    for core in range(N_CORES):
        xs = x[core * TOK_PER_CORE:(core + 1) * TOK_PER_CORE]
        m = {"xb11": pack_x(xs, 0, 0), "xb12": pack_x(xs, 0, 512),
             "xb21": pack_x(xs, 2048, 0), "xb22": pack_x(xs, 2048, 512),
             "wt7": wt7, "ct": ct, "ac": ac, "bc": bc}
        in_maps.append(m)
    return in_maps


def _assemble(results):
    parts = []
    for core in range(N_CORES):
        oc = results[core]["out"]  # [32, 2, 128, 512]
        parts.append(oc.transpose(1, 3, 0, 2).reshape(TOK_PER_CORE, OUT_F))
    out = np.concatenate(parts, axis=0)
    return out.reshape(4, 2048, OUT_F).astype(np.float32)


def kernel(input, weight, bias, centroids, assignments):
    from concourse.bass_utils import run_bass_kernel_spmd

    nc = _get_nc()
    in_maps = _prep_in_maps(input, weight, bias, centroids, assignments)
    res = run_bass_kernel_spmd(nc, in_maps, core_ids=list(range(N_CORES)))
    return _assemble(res.results)


# revision 25
# speedup vs baseline: 1.0706x; 1.0706x over previous
"""HKLinear Trainium2 kernel — 8-core SPMD over tokens, one-level Strassen.

OUT.T = W @ x.T with W [4096f, 4096k], x.T [4096k, 1024t] per core, split
2x2 over (f, k) for W / (k, t) for x.T:
    P1=(A11+A22)(B11+B22) P2=(A21+A22)B11 P3=A11(B12-B22) P4=A22(B21-B11)
    P5=(A11+A12)B22 P6=(A21-A11)(B11+B12) P7=(A12-A22)(B21+B22)
    C11=P1+P4-P5+P7  C12=P3+P5  C21=P2+P4  C22=P1-P2+P3+P6
7/8 of the direct matmul cycles (1792 vs 2048 MMs); W-combos are precomputed
on the host (linear prep of weights), x-combos on the DVE, recombination on
the DVE from PSUM.  Measured rel err 5.7e-3 on hw (gate 2e-2).
"""

import numpy as np
import ml_dtypes

N_CORES = 8
IN_F = 4096
OUT_F = 4096
N_CLUSTERS = 64
THRESHOLD = 0.01
TEMPERATURE = 0.1
N_TOKENS = 8192
TOK_PER_CORE = 1024

KTH = 16          # k-tiles per K-half
NTH = 16          # feature chunks per F-half
P2_HEAD_J = 7     # P2 chunks computed in the head
PARK_J = 4        # j's recombined unmasked (mask applied later)
EXP_SHIFT = -30.0

BF16 = ml_dtypes.bfloat16


def _install_ldw_dedup():
    import concourse.tile as tile
    import concourse.mybir as mybir
    if getattr(tile, "_ldw_dedup_installed", False):
        return
    orig_legalize = tile.tile_legalize

    def ldw_key(i):
        return (str(i.ins[0]), str(i.perf_mode), str(i.tile_position),
                str(i.tile_size), str(i.is_transpose))

    def dedup_legalize(ordered, nc):
        out = orig_legalize(ordered, nc)
        for bb, insts in out.items():
            new, last_key, pending = [], None, None
            for i in insts:
                if getattr(i, "engine", None) != mybir.EngineType.PE:
                    new.append(i)
                    continue
                nm = type(i).__name__
                if nm == "InstLdweights":
                    k = ldw_key(i)
                    if k == last_key:
                        pending = i
                        continue
                    last_key = k
                    new.append(i)
                elif nm == "InstMatmult":
                    if pending is not None:
                        i.merge_dependencies_from(pending)
                        pending = None
                    new.append(i)
                else:
                    last_key, pending = None, None
                    new.append(i)
            out[bb] = new
        return out

    tile.tile_legalize = dedup_legalize
    tile._ldw_dedup_installed = True


def _build_bass():
    _install_ldw_dedup()
    import concourse.bass as bass
    import concourse.mybir as mybir
    import concourse.tile as tile
    from concourse import bacc
    from concourse.bass import ds

    f32 = mybir.dt.float32
    bf16 = mybir.dt.bfloat16
    AT = mybir.AluOpType

    nc = bacc.Bacc("TRN2", target_bir_lowering=False, debug=False,
                   num_devices=N_CORES)

    # W combos M1..M7, packed [i][j][p(k%128)][kt][f]
    wt7_d = nc.dram_tensor("wt7", [7, NTH, 128, KTH, 128], bf16, kind="ExternalInput")
    xb_d = {b: nc.dram_tensor(f"x{b}", [128, KTH, 512], bf16, kind="ExternalInput")
            for b in ("b11", "b12", "b21", "b22")}
    ct_d = nc.dram_tensor("ct", [128, 2 * KTH, N_CLUSTERS], bf16, kind="ExternalInput")
    ac_d = nc.dram_tensor("ac", [N_CLUSTERS, 32, 128], bf16, kind="ExternalInput")
    bc_d = nc.dram_tensor("bc", [128, 32], f32, kind="ExternalInput")
    out_d = nc.dram_tensor("out", [32, 2, 128, 512], f32, kind="ExternalOutput")

    with tile.TileContext(nc) as tc:
        with (
            tc.tile_pool(name="resident", bufs=1) as resident,
            tc.tile_pool(name="xc", bufs=5) as xc,
            tc.tile_pool(name="wpool", bufs=7) as wpool,
            tc.tile_pool(name="p2pool", bufs=P2_HEAD_J) as p2pool,
            tc.tile_pool(name="tmp", bufs=3) as tmp,
            tc.tile_pool(name="cpark", bufs=4 * PARK_J) as cpark,
            tc.tile_pool(name="opool", bufs=2) as opool,
            tc.tile_pool(name="route_sb", bufs=1) as route_sb,
            tc.tile_pool(name="pmain", bufs=7, space="PSUM") as pmain,
            tc.tile_pool(name="psum_scr", bufs=1, space="PSUM") as psum_scr,
            tc.tile_pool(name="cc_dram", bufs=1, space="DRAM") as cc_dram,
        ):
            # ---- DMA order: ct, head W combos, x blocks chunked, rest ----
            ct_sb = resident.tile([128, 2 * KTH, N_CLUSTERS], bf16)
            nc.sync.dma_start(ct_sb[:], ct_d[:])

            w2_head = []
            for j in range(5):
                w_sb = wpool.tile([128, KTH, 128], bf16, tag="w_sb",
                                  name=f"w2h_{j}")
                nc.sync.dma_start(w_sb[:], wt7_d[1, j])
                w2_head.append(w_sb)

            xb11 = resident.tile([128, KTH, 512], bf16)
            late_w = {6: 5, 8: 6}  # chunk idx -> head W combo to slip in
            for k in range(KTH):
                nc.sync.dma_start(xb11[:, k], xb_d["b11"][:, k])
                if k in late_w:
                    j = late_w[k]
                    w_sb = wpool.tile([128, KTH, 128], bf16, tag="w_sb",
                                      name=f"w2h_{j}")
                    nc.sync.dma_start(w_sb[:], wt7_d[1, j])
                    w2_head.append(w_sb)
            xb21 = xc.tile([128, KTH, 512], bf16, tag="xc", name="xb21")
            for k in range(KTH):
                nc.sync.dma_start(xb21[:, k], xb_d["b21"][:, k])
            xb12 = xc.tile([128, KTH, 512], bf16, tag="xc", name="xb12")
            for k in range(KTH):
                nc.sync.dma_start(xb12[:, k], xb_d["b12"][:, k])
            xb22 = resident.tile([128, KTH, 512], bf16)
            for k in range(KTH):
                nc.sync.dma_start(xb22[:, k], xb_d["b22"][:, k])
            a_sb = resident.tile([N_CLUSTERS, 32, 128], bf16)
            nc.sync.dma_start(a_sb[:], ac_d[:])
            bc_sb = resident.tile([128, 32], f32)
            nc.sync.dma_start(bc_sb[:], bc_d[:])

            shift_col = route_sb.tile([N_CLUSTERS, 1], f32)
            nc.vector.memset(shift_col[:], EXP_SHIFT)
            ones_c = route_sb.tile([N_CLUSTERS, 1], bf16)
            nc.vector.memset(ones_c[:], 1.0)
            ones_r = route_sb.tile([1, N_CLUSTERS], bf16)
            nc.vector.memset(ones_r[:], 1.0)
            ms_sb = route_sb.tile([128, 32], f32)
            bmask_sb = route_sb.tile([128, 32], f32)

            # ---- head: P2[j] for j<P2_HEAD_J, paced with the b11 stream ----
            p2sb = []
            for jg in (range(0, 5), range(5, P2_HEAD_J)):
                ps = {j: pmain.tile([128, 512], f32, tag="pm", name=f"p2_{j}")
                      for j in jg}
                for k in range(KTH):
                    for j in jg:
                        nc.tensor.matmul(ps[j][:], w2_head[j][:, k, :],
                                         xb11[:, k, :],
                                         start=(k == 0), stop=(k == KTH - 1))
                for j in jg:
                    s_t = p2pool.tile([128, 512], bf16, tag="p2s")
                    nc.scalar.activation(s_t[:], ps[j][:],
                                         mybir.ActivationFunctionType.Copy)
                    p2sb.append(s_t)

            # ---- routing on raw x blocks ----
            psum_ls = [pmain.tile([128, 512], f32, tag="pm", name=f"rl_{h}")
                       for h in range(2)]
            rhs_half = {0: (xb11, xb21), 1: (xb12, xb22)}
            for k in range(2 * KTH):
                for h in range(2):
                    lo, hi = rhs_half[h]
                    rhs = lo[:, k, :] if k < KTH else hi[:, k - KTH, :]
                    nc.tensor.matmul(psum_ls[h][ds(0, N_CLUSTERS), :],
                                     ct_sb[:, k, :], rhs,
                                     start=(k == 0), stop=(k == 2 * KTH - 1))
            cmax_h = []
            for h in range(2):
                pl = psum_ls[h][ds(0, N_CLUSTERS), :]
                e_f = route_sb.tile([N_CLUSTERS, 512], f32, tag="e_f", bufs=1)
                nc.scalar.activation(e_f[:], pl,
                                     mybir.ActivationFunctionType.Exp,
                                     bias=shift_col[:], scale=1.0)
                e_b = route_sb.tile([N_CLUSTERS, 512], bf16, tag="e_b", bufs=1)
                nc.scalar.activation(e_b[:], pl,
                                     mybir.ActivationFunctionType.Exp,
                                     bias=shift_col[:], scale=1.0)
                psum_s = psum_scr.tile([128, 512], f32, tag="scr")
                nc.tensor.matmul(psum_s[ds(0, 1), :], ones_c[:], e_b[:],
                                 start=True, stop=True)
                s_b = route_sb.tile([1, 512], bf16, tag="s_b", bufs=1)
                nc.scalar.activation(s_b[:], psum_s[ds(0, 1), :],
                                     mybir.ActivationFunctionType.Copy,
                                     scale=THRESHOLD)
                psum_b = psum_scr.tile([128, 512], f32, tag="scr")
                nc.tensor.matmul(psum_b[ds(0, N_CLUSTERS), :], ones_r[:], s_b[:],
                                 start=True, stop=True)
                d_sb = route_sb.tile([N_CLUSTERS, 512], f32, tag="d_sb", bufs=1)
                nc.vector.tensor_tensor(d_sb[:], e_f[:],
                                        psum_b[ds(0, N_CLUSTERS), :],
                                        op=AT.subtract)
                cm = route_sb.tile([N_CLUSTERS, 1], f32, tag="cm", bufs=2)
                nc.vector.reduce_max(cm[:], d_sb[:], axis=mybir.AxisListType.X)
                cmax_h.append(cm)
            cmax = route_sb.tile([N_CLUSTERS, 1], f32)
            nc.vector.tensor_tensor(cmax[:], cmax_h[0][:], cmax_h[1][:], op=AT.max)

            cc_in = cc_dram.tile([N_CLUSTERS, 1], f32)
            cc_out = cc_dram.tile([N_CLUSTERS, 1], f32, addr_space="Shared")
            nc.gpsimd.dma_start(cc_in[:], cmax[:])
            nc.gpsimd.collective_compute(
                "AllReduce", AT.max,
                replica_groups=[list(range(N_CORES))],
                ins=[cc_in.opt()], outs=[cc_out.opt()],
            )
            cmax_red = route_sb.tile([N_CLUSTERS, 1], f32)
            nc.gpsimd.dma_start(cmax_red[:], cc_out[:])
            sel_f = route_sb.tile([N_CLUSTERS, 1], f32)
            nc.vector.tensor_scalar(sel_f[:], cmax_red[:], 0.0, None,
                                    op0=AT.is_gt)
            sel_bf = route_sb.tile([N_CLUSTERS, 1], bf16)
            nc.vector.tensor_copy(sel_bf[:], sel_f[:])

            # ---- x combos on DVE.  xc is a 5-slot ring with allocation order
            # xb21, xb12, R1, R4, R7, R3, R6: R3 lands in xb21's slot (its
            # inputs are xb12/xb22) and R6 = R1 + R3 lands in xb12's slot
            # (algebraic identity B11+B12 = (B11+B22) + (B12-B22), avoiding a
            # read of the xb12 memory it overwrites).
            def combo(name, a, b, op):
                t = xc.tile([128, KTH, 512], bf16, tag="xc", name=name)
                for k in range(KTH):
                    nc.vector.tensor_tensor(t[:, k], a[:, k], b[:, k], op=op)
                return t
            R1 = combo("R1", xb11, xb22, AT.add)
            R4 = combo("R4", xb21, xb11, AT.subtract)
            R7 = combo("R7", xb21, xb22, AT.add)
            R3 = combo("R3", xb12, xb22, AT.subtract)
            R6 = combo("R6", R1, R3, AT.add)

            # product index -> (wt7 row, rhs tile)
            prod_rhs = {1: R1, 2: xb11, 3: R3, 4: R4, 5: xb22, 6: R6, 7: R7}

            def emit_product(i, j):
                w_sb = wpool.tile([128, KTH, 128], bf16, tag="w_sb",
                                  name=f"w_{i}_{j}")
                nc.sync.dma_start(w_sb[:], wt7_d[i - 1, j])
                ps = pmain.tile([128, 512], f32, tag="pm", name=f"p_{i}_{j}")
                rhs = prod_rhs[i]
                for k in range(KTH):
                    nc.tensor.matmul(ps[:], w_sb[:, k, :], rhs[:, k, :],
                                     start=(k == 0), stop=(k == KTH - 1))
                return ps

            def stage(ps, name):
                # ACT-drain a product psum to a bf16 SBUF tile (frees the bank)
                s_t = tmp.tile([128, 512], bf16, tag="pstg", bufs=8, name=name)
                nc.scalar.activation(s_t[:], ps[:],
                                     mybir.ActivationFunctionType.Copy)
                return s_t

            def chain(parts, masked_n):
                # parts: (sign, AP) with at most the LAST one living in PSUM
                acc = None
                for sgn, ap in parts:
                    if acc is None:
                        assert sgn > 0
                        acc = ap
                        continue
                    t = tmp.tile([128, 512], f32, tag="t")
                    nc.vector.tensor_tensor(t[:], acc, ap,
                                            op=(AT.add if sgn > 0 else AT.subtract))
                    acc = t[:]
                if masked_n is None:
                    o = cpark.tile([128, 512], bf16, tag="cp")
                    nc.vector.tensor_copy(o[:], acc)
                    return o
                o = opool.tile([128, 512], f32, tag="o_sb")
                nc.vector.tensor_scalar(o[:], acc,
                                        ms_sb[:, ds(masked_n, 1)],
                                        bmask_sb[:, ds(masked_n, 1)],
                                        op0=AT.mult, op1=AT.add)
                return o

            parked = []

            def finish(acc, masked_n):
                if masked_n is None:
                    o = cpark.tile([128, 512], bf16, tag="cp")
                    nc.vector.tensor_copy(o[:], acc)
                    return o
                o = opool.tile([128, 512], f32, tag="o_sb")
                nc.vector.tensor_scalar(o[:], acc,
                                        ms_sb[:, ds(masked_n, 1)],
                                        bmask_sb[:, ds(masked_n, 1)],
                                        op0=AT.mult, op1=AT.add)
                return o

            def tt(a, b, op):
                t = tmp.tile([128, 512], f32, tag="t", bufs=4)
                nc.vector.tensor_tensor(t[:], a, b, op=op)
                return t[:]

            def emit_j(j):
                # chains interleave with the product matmuls: everything that
                # doesn't need P6/P7 is emitted before their MMs, so only one
                # tensor_tensor (+mask) trails the last matmul of the group
                masked = j >= PARK_J
                if j < P2_HEAD_J:
                    p2 = p2sb[j][:]
                else:
                    p2 = stage(emit_product(2, j), f"p2s_{j}")[:]
                p5 = stage(emit_product(5, j), f"p5s_{j}")[:]
                p1 = stage(emit_product(1, j), f"p1s_{j}")[:]
                p4 = stage(emit_product(4, j), f"p4s_{j}")[:]
                n1, n2 = j, NTH + j
                c21 = finish(tt(p2, p4, AT.add), n2 if masked else None)
                t_b = tt(tt(p1, p4, AT.add), p5, AT.subtract)      # c11 partial
                p3 = stage(emit_product(3, j), f"p3s_{j}")[:]
                c12 = finish(tt(p3, p5, AT.add), n1 if masked else None)
                t_d = tt(tt(p1, p2, AT.subtract), p3, AT.add)      # c22 partial
                p6 = emit_product(6, j)[:]   # single-read, stays in PSUM
                c22 = finish(tt(t_d, p6, AT.add), n2 if masked else None)
                p7 = emit_product(7, j)[:]   # single-read, stays in PSUM
                c11 = finish(tt(t_b, p7, AT.add), n1 if masked else None)
                tiles = {(n1, 0): c11, (n2, 0): c21, (n1, 1): c12, (n2, 1): c22}
                if masked:
                    for (n, h), t in tiles.items():
                        nc.sync.dma_start(out_d[n, h], t[:])
                else:
                    parked.append(tiles)

            for j in range(PARK_J):
                emit_j(j)

            # ---- row mask (PE gather), then flush parked chunks ----
            psum_m = psum_scr.tile([128, 512], f32, tag="scr")
            for n in range(32):
                nc.tensor.matmul(psum_m[:, ds(n, 1)], a_sb[:, n, :], sel_bf[:],
                                 start=True, stop=True)
            nc.scalar.activation(ms_sb[:], psum_m[:, ds(0, 32)],
                                 mybir.ActivationFunctionType.Copy)
            nc.vector.tensor_tensor(bmask_sb[:], ms_sb[:], bc_sb[:], op=AT.mult)

            for tiles in parked:
                for (n, h), t in tiles.items():
                    o = opool.tile([128, 512], f32, tag="o_sb")
                    nc.vector.tensor_scalar(o[:], t[:],
                                            ms_sb[:, ds(n, 1)],
                                            bmask_sb[:, ds(n, 1)],
                                            op0=AT.mult, op1=AT.add)
                    nc.sync.dma_start(out_d[n, h], o[:])

            for j in range(PARK_J, NTH):
                emit_j(j)

    nc.compile()
    return nc


_NC_CACHE = None


def _get_nc():
    global _NC_CACHE
    if _NC_CACHE is None:
        _NC_CACHE = _build_bass()
    return _NC_CACHE


def _prep_in_maps(input, weight, bias, centroids, assignments):
    x = np.ascontiguousarray(np.asarray(input, dtype=np.float32).reshape(N_TOKENS, IN_F))
    w = np.asarray(weight, dtype=np.float32)
    b = np.asarray(bias, dtype=np.float32)
    c = np.asarray(centroids, dtype=np.float32)
    a = np.asarray(assignments)

    A11, A12 = w[:2048, :2048], w[:2048, 2048:]
    A21, A22 = w[2048:, :2048], w[2048:, 2048:]
    Ms = [A11 + A22, A21 + A22, A11, A22, A11 + A12, A21 - A11, A12 - A22]

    def pack_m(m):
        # m [2048f, 2048k] -> [NTH, 128p, KTH, 128f]
        return m.reshape(NTH, 128, KTH, 128).transpose(0, 3, 2, 1)
    wt7 = np.ascontiguousarray(np.stack([pack_m(m) for m in Ms])).astype(BF16)

    ct = np.ascontiguousarray(
        (c / TEMPERATURE).T.reshape(2 * KTH, 128, N_CLUSTERS).transpose(1, 0, 2)).astype(BF16)
    ac = (a[None, :] == np.arange(N_CLUSTERS, dtype=a.dtype)[:, None])
    ac = np.ascontiguousarray(ac.reshape(N_CLUSTERS, 32, 128)).astype(BF16)
    bc = np.ascontiguousarray(b.reshape(32, 128).T).astype(np.float32)

    def pack_x(xs, klo, tlo):
        # -> [128p, KTH, 512t]: blk[p, kt, t] = xs[tlo+t, klo + kt*128 + p]
        blk = xs[tlo:tlo + 512, klo:klo + 2048]
        return np.ascontiguousarray(
            blk.T.reshape(KTH, 128, 512).transpose(1, 0, 2)).astype(BF16)

    in_maps = []
    for core in range(N_CORES):
        xs = x[core * TOK_PER_CORE:(core + 1) * TOK_PER_CORE]
        m = {"xb11": pack_x(xs, 0, 0), "xb12": pack_x(xs, 0, 512),
             "xb21": pack_x(xs, 2048, 0), "xb22": pack_x(xs, 2048, 512),
             "wt7": wt7, "ct": ct, "ac": ac, "bc": bc}
        in_maps.append(m)
    return in_maps


def _assemble(results):
    parts = []
    for core in range(N_CORES):
        oc = results[core]["out"]  # [32, 2, 128, 512]
        parts.append(oc.transpose(1, 3, 0, 2).reshape(TOK_PER_CORE, OUT_F))
    out = np.concatenate(parts, axis=0)
    return out.reshape(4, 2048, OUT_F).astype(np.float32)


def kernel(input, weight, bias, centroids, assignments):
    from concourse.bass_utils import run_bass_kernel_spmd

    nc = _get_nc()
    in_maps = _prep_in_maps(input, weight, bias, centroids, assignments)
    res = run_bass_kernel_spmd(nc, in_maps, core_ids=list(range(N_CORES)))
    return _assemble(res.results)
